# revision 14
# baseline (speedup 1.0000x reference)
"""CombinePatches (3D col2im fold + overlap-count normalize) on 8 TRN2 NeuronCores.

Decomposition (validated numerically against the reference):
  out[b, 2q+kd, 2s+kh, 2u+kw, c] (+)= patches[b, q, s, u, kd, kh, kw, c], then
  out /= cnt, cnt = cd(d)*ch(h)*cw(w) separable overlap counts.

Sharding: 8 cores = B(2) x D-chunks(4). Each core computes 16 output d-rows from
9 od-slices of patches (1 halo slice, zero-padded at global edges by the host).

Per core, per output row d (r=d%2, q=d//2):
  - DVE w-fold: T[s, j, w, c] = A[s, floor(w/2), j, ...] + A[s, floor(w/2)-1, ...]
    done for A = slice q (kd=r) and B = slice q-1 (kd=r+2), with the ow dim
    pre-split into two halves on partitions (p = uhalf*64 + s) so each DVE op
    uses all 128 lanes.
  - TensorE h-fold: O[h, (w,c)] = sum_j Mh_j^T @ T_j accumulated in PSUM over
    (j x {A,B} x {w-half}) = 16 float32r matmuls; 0.25*rh(h) baked into Mh
    (0.25 = interior rd * interior rw).
  - ScalarE eviction: PSUM -> SBUF copy, then DMA store on the scalar ring.
Host fixes the global d-edge rows and w-edge columns by x2 after gather.
"""
import sys

for _p in ("/opt/trn_rl_repo", "/opt/trn_rl_repo/pypackages"):
    if _p not in sys.path:
        sys.path.insert(0, _p)

from contextlib import ExitStack

import numpy as np

import concourse.bass as bass
import concourse.tile as tile
from concourse import bacc, mybir
from concourse import bass_utils

B, D, H, W, C = 2, 64, 128, 128, 4
od, oh, ow = 31, 63, 63
NS, X = 9, 33       # od-slices per core (incl 1 halo), padded u-slots per half
RPC = 16            # output d-rows per core
MM_DT = mybir.dt.bfloat16
import ml_dtypes

BF16 = ml_dtypes.bfloat16

_cache = {}


def _build():
    nc = bacc.Bacc(
        "TRN2",
        target_bir_lowering=False,
        debug=False,
        enable_asserts=False,
        num_devices=8,
    )
    # flat pp: [half-slice k=0 (kd 2,3 only)] + [7 full slices] + [half k=8 (kd 0,1)]
    HALF_F, FULL_F = X * 128, X * 256
    PP_TOTAL = 128 * (2 * HALF_F + 7 * FULL_F)
    pp_d = nc.dram_tensor(
        "pp", [PP_TOTAL], MM_DT, kind="ExternalInput"
    ).ap()
    wm_d = nc.dram_tensor("wm", [128, 1024], MM_DT, kind="ExternalInput").ap()
    out_d = nc.dram_tensor(
        "out", [RPC, H, W, C], MM_DT, kind="ExternalOutput"
    ).ap()

    with ExitStack() as ctx:
        tc = ctx.enter_context(tile.TileContext(nc))
        const_pool = ctx.enter_context(tc.tile_pool(name="const", bufs=1))
        # bufs=6: enough slot slack that compute hiccups don't stall the load
        # stream, but not so many outstanding DMAs that packets slow down
        # (9 outstanding loads measurably degraded early HBM throughput).
        slice_pool = ctx.enter_context(tc.tile_pool(name="slice", bufs=6))
        t_pool = ctx.enter_context(tc.tile_pool(name="tt", bufs=6))
        ev_pool = ctx.enter_context(tc.tile_pool(name="ev", bufs=3))
        psum_pool = ctx.enter_context(tc.tile_pool(name="ps", bufs=3, space="PSUM"))

        # constants go on the scalar-engine HWDGE ring so the sync ring is
        # purely slice loads (HWDGE rings are FIFO per issuing engine).
        wm_sb = const_pool.tile([128, 1024], MM_DT)
        nc.scalar.dma_start(wm_sb[:], wm_d[:])

        def slice_region(k):
            """(flat offset, free width, n_kd, kd_base) of slice k in pp."""
            if k == 0:
                return 0, HALF_F, 2, 2
            if k == NS - 1:
                return 128 * (HALF_F + 7 * FULL_F), HALF_F, 2, 0
            return 128 * (HALF_F + (k - 1) * FULL_F), FULL_F, 4, 0

        tiles = {}
        for k in range(NS):
            off, fw, nkd, kdb = slice_region(k)
            t = slice_pool.tile([128, fw], MM_DT, tag="slice")
            src = pp_d[off : off + 128 * fw].rearrange("(p f) -> p f", f=fw)
            nc.sync.dma_start(t[:], src)
            tiles[k] = (t, nkd, kdb)
            if k == 0:
                continue
            # one PSUM tile (2 banks), one eviction, one store per slice
            # (= 2 output rows): fewer DMAs and semaphores shrink both the
            # serialized scalar work and the fixed end-of-NEFF sem-drain.
            ps = psum_pool.tile([128, 1024], mybir.dt.float32, tag="ps")
            for rr in range(2):
                TA = t_pool.tile([128, 1024], MM_DT, tag="T")
                TB = t_pool.tile([128, 1024], MM_DT, tag="T")
                for T, (tk, t_nkd, t_kdb), kd in (
                    (TA, tiles[k], rr),
                    (TB, tiles[k - 1], rr + 2),
                ):
                    v = tk[:].rearrange(
                        "p (x kd j v c) -> p x kd j v c", x=X, kd=t_nkd, j=4, v=4, c=4
                    )
                    ki = kd - t_kdb
                    t1 = v[:, 1:33, ki, :, 0:2, :].rearrange("p m j t c -> p j m t c")
                    t2 = v[:, 0:32, ki, :, 2:4, :].rearrange("p m j t c -> p j m t c")
                    To = T[:].rearrange("p (j m t c) -> p j m t c", j=4, m=32, t=2, c=4)
                    nc.vector.tensor_add(To, t1, t2)
                # pre-sum the two kd contributions on DVE (contiguous op) so
                # the PSUM accumulation needs 8 matmuls/row, not 16: PE rhs
                # reads starve against saturated DMA SBUF writes (~2x slower
                # matmul), so tensor must stay well under the DMA rate.
                nc.vector.tensor_add(TA[:], TA[:], TB[:])
                for half in range(2):
                    outseg = ps[:, rr * 512 + half * 256 : rr * 512 + (half + 1) * 256]
                    for j in range(4):
                        # K=128 with zero-padded block-diagonal weights keeps
                        # every matmul at tile_position (0,0): mixing PE tile
                        # positions in one NEFF hangs at runtime.
                        lhsT = wm_sb[:, 512 * half + j * 128 : 512 * half + (j + 1) * 128]
                        rhs = TA[:, j * 256 : (j + 1) * 256]
                        nc.tensor.matmul(
                            outseg, lhsT, rhs, start=(j == 0), stop=(j == 3)
                        )
            # evict on ScalarE: evictions wait on matmuls, and in the DVE
            # FIFO they would delay later w-folds, which gate slice loads
            # via slot release. rw's interior 0.5 is folded into wm; the
            # host rescales the 4 global w-edge columns.
            ev = ev_pool.tile([128, 1024], MM_DT, tag="ev")
            nc.scalar.copy(ev[:], ps[:])
            # stores on the scalar ring: a store waiting on eviction must
            # not head-of-line-block the next slice load on the sync ring
            d0 = 2 * (k - 1)
            nc.scalar.dma_start(
                out_d[d0 : d0 + 2].rearrange("d h w c -> h d (w c)"),
                ev[:].rearrange("p (d f) -> p d f", d=2),
            )
    nc.compile()
    return nc


def _host_tables():
    rh = np.where(
        (np.arange(H) < 2) | (np.arange(H) >= H - 2), 1.0, 0.5
    ).astype(np.float32)
    # [half*64+s, whalf*512 + j*128 + h], block-diagonal in (half, whalf).
    # 0.25 = interior rd (0.5) * interior rw (0.5); host rescales d/w edges.
    wm = np.zeros((128, 1024), np.float32)
    s_idx = np.arange(oh)
    for j in range(4):
        h = 2 * s_idx + j
        wm[s_idx, j * 128 + h] = 0.25 * rh[h]
        wm[64 + s_idx, 512 + j * 128 + h] = 0.25 * rh[h]
    return wm.astype(BF16)


def _shard_inputs(patches):
    """Build per-core flat patch blocks: half k=0 (kd 2,3) + 7 full + half k=8
    (kd 0,1), each region [128 partitions x freewidth] flattened p-major."""
    P5 = np.ascontiguousarray(patches).reshape(B, od, oh, ow, 256).astype(BF16)
    # q-slot k = q+1 for q in [-1, 32); u-slot x = u+1 for u in [-1, 65)
    Pu = np.zeros((B, od + 2, 64, 66, 256), BF16)
    Pu[:, 1 : od + 1, 0:oh, 1 : ow + 1, :] = P5
    pps = []
    for core in range(8):
        b, kc = core // 4, core % 4
        s0 = 8 * kc  # = qbase + 1
        # [NS, 2(uhalf), 64(s), X, 256]
        pp = np.stack(
            [Pu[b, s0 : s0 + NS, :, 0:X, :], Pu[b, s0 : s0 + NS, :, 32 : 32 + X, :]],
            axis=1,
        )
        parts = [
            np.ascontiguousarray(pp[0, :, :, :, 128:256]).reshape(-1),  # kd 2,3
            np.ascontiguousarray(pp[1 : NS - 1]).reshape(-1),
            np.ascontiguousarray(pp[NS - 1, :, :, :, 0:128]).reshape(-1),  # kd 0,1
        ]
        pps.append(np.concatenate(parts))
    return pps


def _run(patches, trace=False):
    if "nc" not in _cache:
        _cache["nc"] = _build()
        _cache["tables"] = _host_tables()
    nc = _cache["nc"]
    wm = _cache["tables"]
    pps = _shard_inputs(np.asarray(patches, dtype=np.float32))
    in_maps = [{"pp": pps[core], "wm": wm} for core in range(8)]
    res = bass_utils.run_bass_kernel_spmd(
        nc, in_maps, core_ids=list(range(8)), trace=trace
    )
    out = np.zeros((B, D, H, W, C), np.float32)
    for core in range(8):
        b, kc = core // 4, core % 4
        out[b, RPC * kc : RPC * (kc + 1)] = np.asarray(
            res.results[core]["out"]
        ).astype(np.float32)
    out[:, [0, 1, D - 2, D - 1]] *= 2.0
    out[:, :, :, [0, 1, W - 2, W - 1], :] *= 2.0
    return out, res


def kernel(patches, inputs):
    out, _ = _run(patches)
    return out

